# revision 2
# baseline (speedup 1.0000x reference)
"""Causal self-attention (B=4, T=2048, D=1024, H=16) on 8 Trainium2 cores, v2.

Sharding: tensor-parallel over heads - 2 heads per core. Each core computes
its QKV shard, causal attention for its heads, and a partial output
projection; the host sums the 8 partials.

v2 changes vs baseline:
  - bf16 everywhere off-chip and for matmul operands (fp32 PSUM accum):
    halves DMA traffic; diagonal matmuls avoid the fp32r small-moving 4x.
  - 1024-wide q-chunks: half the score-matmul/exp instructions.
  - causal mask is a single [128,128] lower-tri 0/1 bf16 tile applied
    multiplicatively to the exp'd diagonal block on DVE (was: additive
    NEG mask + per-column mask tensor on fp32).
  - out written as bf16 (host sums partials in fp32).
  - batched DMAs (one per x chunk, one per 512-row output block).

Per-core dataflow (all matmuls bf16 in / fp32 PSUM out):
  phase 1 per 1024-tok chunk: xts DMA; q,k matmuls (contract d on
      partitions, moving 1024) -> qT/kT [feat, tok] bf16 SBUF; v feature-
      layout matmuls + PE transposes -> vv [k, kt, h, 0:64] bf16 with ones
      in cols 64:128 (row-sum trick).
  phase 2 per (1024-chunk, head): per k-tile one scores^T matmul
      [k,live-q] into a [128,1024] PSUM tile; exp with 1/8 scale
      (ACT, PSUM->SBUF bf16); diagonal 128x128 block masked in-place
      (DVE mult by tri mask); AV accumulation per 512-half with ones
      rows giving denominators on partitions 64:127; reciprocal * mult
      -> attnT bf16.
  phase 3 per 512-tok block: out projection attnT^T x wout -> [tok, 1024]
      PSUM, copy to bf16 ob, one DMA per block. Host sums partials.
"""

import os
import sys

sys.path.insert(0, "/opt/trn_rl_repo")

import numpy as np
import ml_dtypes
from contextlib import ExitStack

import concourse.bass as bass
import concourse.mybir as mybir
import concourse.tile as tile
from concourse import bacc
from concourse.bass_utils import run_bass_kernel_spmd

B, T, D, H, HD = 4, 2048, 1024, 16, 64
NCORES = 8
HPC = H // NCORES          # heads per core = 2
DC = HPC * HD              # per-core feature width = 128
TOK = B * T                # 8192
TB = T // 128              # k-tiles per batch = 16
CW = 1024                  # q-chunk width
NCH = T // CW              # chunks per batch = 2
F32 = mybir.dt.float32
F32R = mybir.dt.float32r
BF16 = mybir.dt.bfloat16
EXP = mybir.ActivationFunctionType.Exp
SCALE = 1.0 / 8.0          # 1/sqrt(HD)

LAST_RESULTS = None


def _env(name, dflt):
    return os.environ.get(name, dflt)


QK_EV = _env("K_QK_EV", "s")       # qT/kT PSUM->SBUF copies: s=ACT, v=DVE
OB_EV = _env("K_OB_EV", "v")       # outproj copies: v=DVE, s=ACT, b=alternate
V_EV = _env("K_V_EV", "v")         # v transpose-pack copies: v=DVE, s=ACT
MASK_EV = _env("K_MASK_EV", "v")   # tri-mask mult: v=DVE, p=Pool
OUTDMA_EV = _env("K_OUTDMA_EV", "s")  # out DMA issue queue: p=Pool, s=SP
AV_DT = _env("K_AV_DT", "r")       # probs/V dtype: r=fp32r, b=bf16


def _copy(nc, ev, out, in_):
    if ev == "v":
        nc.vector.tensor_copy(out, in_)
    elif ev == "p":
        nc.gpsimd.tensor_copy(out, in_)
    else:
        nc.scalar.copy(out, in_)


def _attention_kernel(tc, out4, xTr, wqkvT, woutT, trimaskd, identd, vonesd):
    nc = tc.nc
    with ExitStack() as ctx:
        const = ctx.enter_context(tc.tile_pool(name="const", bufs=1))
        sbqk = ctx.enter_context(tc.tile_pool(name="sbqk", bufs=2))
        sbvv = ctx.enter_context(tc.tile_pool(name="sbvv", bufs=1))
        sbvt = ctx.enter_context(tc.tile_pool(name="sbvt", bufs=2))
        sbx = ctx.enter_context(tc.tile_pool(name="sbx", bufs=2))
        sbpt = ctx.enter_context(tc.tile_pool(name="sbpt", bufs=12))
        sba = ctx.enter_context(tc.tile_pool(name="sba", bufs=2))
        sbrc = ctx.enter_context(tc.tile_pool(name="sbrc", bufs=2))
        sbob = ctx.enter_context(tc.tile_pool(name="sbob", bufs=2))
        psS = ctx.enter_context(tc.tile_pool(name="psS", bufs=2, space="PSUM"))
        psAV = ctx.enter_context(tc.tile_pool(name="psAV", bufs=2, space="PSUM"))
        psOP = ctx.enter_context(tc.tile_pool(name="psOP", bufs=2, space="PSUM"))

        # ---- constants ----
        w_sb = const.tile([128, 8, 3 * DC], BF16, tag="wqkv")
        nc.sync.dma_start(out=w_sb, in_=wqkvT.rearrange("(dt p) f -> p dt f", p=128))
        wo_sb = const.tile([128, D], F32R, tag="wout")
        nc.sync.dma_start(out=wo_sb, in_=woutT)
        trimask = const.tile([128, 128], F32 if AV_DT == "r" else BF16,
                             tag="trimask")
        nc.sync.dma_start(out=trimask, in_=trimaskd)
        ident = const.tile([128, 128], F32R if AV_DT == "r" else BF16,
                           tag="ident")
        nc.sync.dma_start(out=ident, in_=identd)

        pools = (sbqk, sbvv, sbvt, sbx, sbpt, sba, sbrc, sbob, psS, psAV, psOP)

        # vv double-buffered manually (batch parity); ones cols written once
        # (memset on the idle Pool engine - keeps startup off the DMA path)
        # vv [128, kt, h, 128]: cols 0:64 V data (rewritten per batch), cols
        # 64:128 ones for the denominator row-sum trick (written once)
        vvs = []
        for pb in range(2):
            vv = sbvv.tile([128, TB, HPC, 128],
                           F32R if AV_DT == "r" else BF16, tag=f"vv{pb}")
            nc.sync.dma_start(out=vv[:, :, :, 64:128], in_=vonesd.rearrange(
                "p (t h c) -> p t h c", t=TB, h=HPC))
            vvs.append(vv)

        def body():
            _kernel_body(tc, out4, xTr, w_sb, wo_sb, trimask, ident, vvs, pools)

        nloop = int(os.environ.get("K_LOOP", "1"))
        if nloop > 1:
            with tc.For_i(0, nloop, 1):
                body()
        else:
            body()


def _kernel_body(tc, out4, xTr, w_sb, wo_sb, trimask, ident, vvs, pools):
    (sbqk, sbvv, sbvt, sbx, sbpt, sba, sbrc, sbob, psS, psAV, psOP) = pools
    nc = tc.nc

    # x chunk DMAs are issued one batch ahead so the SP queue isn't blocked
    # behind the previous batch's output DMAs when the next batch starts
    xts_tiles = {}

    def fetch_x(b):
        if b >= B:
            return
        for ci in range(NCH):
            tok0 = b * T + ci * CW
            xts = sbx.tile([128, 8, CW], BF16, tag="xts")
            nc.sync.dma_start(out=xts, in_=xTr[:, :, tok0:tok0 + CW])
            xts_tiles[(b, ci)] = xts

    fetch_x(0)
    for b in range(B):
        vv = vvs[b % 2]
        # ================= phase 1: QKV projection =================
        qT = sbqk.tile([128, T], F32R, tag="qT")
        kT = sbqk.tile([128, T], F32R, tag="kT")
        for ci in range(NCH):                    # 1024-token chunks
            xts = xts_tiles.pop((b, ci))
            for ft, dst in ((0, qT), (1, kT)):
                qkp = psS.tile([128, CW], F32, tag="s")
                for hf in range(2):              # matmul moving dim max 512
                    for dt in range(8):
                        nc.tensor.matmul(
                            qkp[:, hf * 512:(hf + 1) * 512],
                            w_sb[:, dt, ft * DC:(ft + 1) * DC],
                            xts[:, dt, hf * 512:(hf + 1) * 512],
                            start=(dt == 0), stop=(dt == 7),
                        )
                _copy(nc, QK_EV, dst[:, ci * CW:(ci + 1) * CW], qkp)
            # v in feature layout then transpose k-tiles of 128 toks
            vp = psS.tile([128, CW], F32, tag="s")
            for hf in range(2):
                for dt in range(8):
                    nc.tensor.matmul(
                        vp[:, hf * 512:(hf + 1) * 512],
                        w_sb[:, dt, 2 * DC:3 * DC],
                        xts[:, dt, hf * 512:(hf + 1) * 512],
                        start=(dt == 0), stop=(dt == 7),
                    )
            vdt = F32R if AV_DT == "r" else BF16
            vT = sbvt.tile([128, CW], vdt, tag="vT")
            _copy(nc, QK_EV, vT, vp)
            for k4 in range(2):
                trp = psOP.tile([128, 4, 128], vdt, tag="op")
                for ki in range(4):
                    kk = k4 * 4 + ki
                    nc.tensor.transpose(
                        trp[:, ki, :],
                        vT[:, kk * 128:(kk + 1) * 128],
                        ident,
                    )
                kt0 = ci * 8 + k4 * 4
                _copy(nc, V_EV,
                      vv[:, kt0:kt0 + 4, :, 0:64],
                      trp.rearrange("p a (h c) -> p a h c", h=HPC))

        fetch_x(b + 1)
        # ====== phase 2 + 3: attention per (chunk, head), fused outproj ====
        attnT = sba.tile([128, T], F32R, tag="attnT")
        for ci in range(NCH):
            nsub = ci * 8                        # fully-live k-tiles
            qs = ci * CW
            all_pts = []
            for h in range(HPC):
                hs = slice(h * 64, (h + 1) * 64)
                pts = []
                for kt in range(nsub):           # sub-diagonal k-tiles
                    sp = psS.tile([128, CW], F32, tag="s")
                    for hf in range(2):
                        nc.tensor.matmul(
                            sp[:, hf * 512:(hf + 1) * 512],
                            kT[hs, kt * 128:(kt + 1) * 128],
                            qT[hs, qs + hf * 512:qs + (hf + 1) * 512],
                            start=True, stop=True,
                        )
                    pt = sbpt.tile([128, CW],
                                   F32R if AV_DT == "r" else BF16, tag="p")
                    nc.scalar.activation(pt, sp, EXP, scale=SCALE)
                    pts.append((kt, pt, 0))
                for o in range(8):               # diagonal-band k-tiles
                    kt = nsub + o
                    off = 128 * o
                    sp = psS.tile([128, CW], F32, tag="s")
                    for c0 in range(off // 512 * 512, CW, 512):
                        o0 = max(off, c0)
                        nc.tensor.matmul(
                            sp[:, o0:c0 + 512],
                            kT[hs, kt * 128:(kt + 1) * 128],
                            qT[hs, qs + o0:qs + c0 + 512],
                            start=True, stop=True,
                        )
                    pt = sbpt.tile([128, CW],
                                   F32R if AV_DT == "r" else BF16, tag="p")
                    nc.scalar.activation(pt[:, off:CW], sp[:, off:CW],
                                         EXP, scale=SCALE)
                    eng = nc.gpsimd if MASK_EV == "p" else nc.vector
                    eng.tensor_tensor(
                        out=pt[:, off:off + 128], in0=pt[:, off:off + 128],
                        in1=trimask, op=mybir.AluOpType.mult)
                    pts.append((kt, pt, off))
                all_pts.append(pts)
            # AV in 512-halves so psav tiles stay one bank; norm right after
            # each half so the next head's AV can reuse the psAV ring
            for h in range(HPC):
                pts = all_pts[h]
                for half in range(2):
                    c0 = half * 512
                    contrib = [(kt, pt, max(off, c0)) for kt, pt, off in pts
                               if max(off, c0) < c0 + 512]
                    avp = psAV.tile([128, 512], F32, tag="av")
                    for i, (kt, pt, o0) in enumerate(contrib):
                        nc.tensor.matmul(
                            avp[:, o0 - c0:512],
                            vv[:, kt, h, :],
                            pt[:, o0:c0 + 512],
                            start=(i == 0), stop=(i == len(contrib) - 1),
                        )
                    rc = sbrc.tile([128, 512], F32, tag="rc")
                    nc.vector.reciprocal(rc[0:64, :], avp[64:128, :])
                    nc.vector.tensor_tensor(
                        out=attnT[h * 64:(h + 1) * 64, qs + c0:qs + c0 + 512],
                        in0=avp[0:64, :], in1=rc[0:64, :],
                        op=mybir.AluOpType.mult,
                    )
            # ---- phase 3: out projection for this chunk's two 512-blocks ---
            for half in range(2):
                qb = ci * 2 + half
                ob = sbob.tile([128, 4, D], BF16, tag="ob")
                for tl in range(4):
                    tt = qb * 4 + tl
                    for fc in range(2):
                        op_ = psOP.tile([128, 512], F32, tag="op")
                        nc.tensor.matmul(
                            op_,
                            attnT[:, tt * 128:(tt + 1) * 128],
                            wo_sb[:, fc * 512:(fc + 1) * 512],
                            start=True, stop=True,
                        )
                        ev = OB_EV if OB_EV != "b" else ("v" if fc == 0 else "s")
                        _copy(nc, ev, ob[:, tl, fc * 512:(fc + 1) * 512], op_)
                if OUTDMA_EV == "p":
                    nc.gpsimd.dma_start(out=out4[b * 4 + qb], in_=ob)
                else:
                    nc.sync.dma_start(out=out4[b * 4 + qb], in_=ob)


def build_module():
    nc = bacc.Bacc("TRN2", target_bir_lowering=False, debug=False,
                   num_devices=NCORES)
    xT = nc.declare_dram_parameter("xT", [D, TOK], BF16, isOutput=False)
    wqkvT = nc.declare_dram_parameter("wqkvT", [D, 3 * DC], BF16, isOutput=False)
    woutT = nc.declare_dram_parameter("woutT", [DC, D], F32R, isOutput=False)
    trimask = nc.declare_dram_parameter(
        "trimask", [128, 128], F32 if AV_DT == "r" else BF16, isOutput=False)
    ident = nc.declare_dram_parameter(
        "ident", [128, 128], F32R if AV_DT == "r" else BF16, isOutput=False)
    vones = nc.declare_dram_parameter(
        "vones", [128, TB * HPC * 64], F32R if AV_DT == "r" else BF16,
        isOutput=False)
    out = nc.declare_dram_parameter("out", [TOK, D], BF16, isOutput=True)
    with tile.TileContext(nc) as tc:
        _attention_kernel(
            tc,
            out[:].rearrange("(n tt p) d -> n p tt d", p=128, tt=4),
            xT[:].rearrange("(dt p) tok -> p dt tok", p=128),
            wqkvT[:], woutT[:], trimask[:], ident[:], vones[:],
        )
    nc.compile()
    return nc


def shard_inputs(x, w_qkv, w_out):
    """Returns per-core input maps (bf16 host-side prep)."""
    bf = ml_dtypes.bfloat16
    x_flat = np.asarray(x, np.float32).reshape(TOK, D)
    xT = np.ascontiguousarray(x_flat.T).astype(bf)       # [D, TOK]
    w_qkv = np.asarray(w_qkv, np.float32)
    w_out = np.asarray(w_out, np.float32)
    kp = np.arange(128)[:, None]
    qf = np.arange(128)[None, :]
    trimask = (kp <= qf).astype(
        np.float32 if AV_DT == "r" else bf)              # [128,128] lower-tri^T
    identm = np.eye(128, dtype=np.float32 if AV_DT == "r" else bf)
    vones = np.ones((128, TB * HPC * 64),
                    np.float32 if AV_DT == "r" else bf)
    in_maps = []
    for c in range(NCORES):
        r0 = c * DC
        wq = w_qkv[r0:r0 + DC]
        wk = w_qkv[D + r0:D + r0 + DC]
        wv = w_qkv[2 * D + r0:2 * D + r0 + DC]
        wqkvT = np.ascontiguousarray(
            np.concatenate([wq, wk, wv], axis=0).T).astype(bf)   # [D, 3*DC]
        woutT = np.ascontiguousarray(w_out[:, r0:r0 + DC].T)     # [DC, D] f32
        in_maps.append({"xT": xT, "wqkvT": wqkvT, "woutT": woutT,
                       "trimask": trimask, "ident": identm, "vones": vones})
    return in_maps


_NC_CACHE = None


def kernel(x, w_qkv, w_out):
    global _NC_CACHE, LAST_RESULTS
    if _NC_CACHE is None:
        _NC_CACHE = build_module()
    nc = _NC_CACHE
    in_maps = shard_inputs(x, w_qkv, w_out)
    os.environ["BASS_NEVER_TRACE"] = "1"
    res = run_bass_kernel_spmd(nc, in_maps, list(range(NCORES)), trace=False)
    LAST_RESULTS = res
    acc = np.zeros((TOK, D), dtype=np.float32)
    for r in res.results:
        acc += r["out"].astype(np.float32)
    return acc.reshape(B, T, D)


# revision 3
# speedup vs baseline: 1.0107x; 1.0107x over previous
"""Causal self-attention (B=4, T=2048, D=1024, H=16) on 8 Trainium2 cores.

Sharding: tensor-parallel over heads - 2 heads per core. Each core computes
its QKV shard, causal attention for its heads, and a partial output
projection; the host sums the 8 partials.

Key choices (vs the fp32 baseline this evolved from):
  - x and w_qkv are bf16 (host-converted) and the partial output is written
    as bf16: ~halves DMA traffic.  QKV matmuls run in bf16; the attention
    matmuls (scores / AV / out-proj) stay fp32r, which self-loads weights
    (bf16 matmuls cost an extra Ldweights sequencer instruction each).
  - 1024-wide q-chunks: one [128,1024] PSUM score tile per k-tile (two
    512-wide matmuls, ISA caps the moving dim at 512) and ONE exp per tile,
    halving the activation-engine instruction count.
  - causal mask is a single [128,128] lower-tri 0/1 tile applied
    multiplicatively in-place to the exp'd diagonal block (DVE), instead of
    an additive -1e9 mask + per-column mask tensor.
  - x chunk DMAs are prefetched one batch ahead so the SP DMA queue is not
    head-of-line blocked behind the previous batch's output DMAs.
  - ones columns of the V tile (denominator row-sum trick) come from one
    strided DMA; the V tile is double-buffered by batch parity so phase 1
    of batch b+1 can overlap attention of batch b.

Per-core dataflow (PSUM accum fp32 everywhere):
  phase 1 per 1024-tok chunk: xts DMA; q,k matmuls (contract d on
      partitions) -> qT/kT [feat, tok] SBUF; v feature-layout matmuls +
      PE transposes -> vv [k, kt, h, 0:64] with ones in cols 64:128.
  phase 2 per (1024-chunk, head): per k-tile one scores^T [k, live-q]
      PSUM tile; exp with fused 1/8 scale (ACT, PSUM->SBUF); diagonal
      128x128 block masked in-place; AV accumulation per 512-half with
      ones rows giving denominators on partitions 64:127; reciprocal *
      mult -> attnT.
  phase 3 per 512-tok block: out projection attnT^T x wout -> [tok, 1024]
      PSUM, copy to bf16 ob, one DMA per block. Host sums partials.
"""

import os
import sys

sys.path.insert(0, "/opt/trn_rl_repo")

import numpy as np
import ml_dtypes
from contextlib import ExitStack

import concourse.bass as bass
import concourse.mybir as mybir
import concourse.tile as tile
from concourse import bacc
from concourse.bass_utils import run_bass_kernel_spmd

B, T, D, H, HD = 4, 2048, 1024, 16, 64
NCORES = 8
HPC = H // NCORES          # heads per core = 2
DC = HPC * HD              # per-core feature width = 128
TOK = B * T                # 8192
TB = T // 128              # k-tiles per batch = 16
CW = 1024                  # q-chunk width
NCH = T // CW              # chunks per batch = 2
F32 = mybir.dt.float32
F32R = mybir.dt.float32r
BF16 = mybir.dt.bfloat16
EXP = mybir.ActivationFunctionType.Exp
SCALE = 1.0 / 8.0          # 1/sqrt(HD)

LAST_RESULTS = None


def _env(name, dflt):
    return os.environ.get(name, dflt)


QK_EV = _env("K_QK_EV", "s")       # qT/kT PSUM->SBUF copies: s=ACT, v=DVE
OB_EV = _env("K_OB_EV", "v")       # outproj copies: v=DVE, s=ACT, b=alternate
V_EV = _env("K_V_EV", "v")         # v transpose-pack copies: v=DVE, s=ACT
MASK_EV = _env("K_MASK_EV", "v")   # tri-mask mult: v=DVE, p=Pool
OUTDMA_EV = _env("K_OUTDMA_EV", "s")  # out DMA issue queue: p=Pool, s=SP
AV_DT = _env("K_AV_DT", "r")       # probs/V dtype: r=fp32r, b=bf16


def _copy(nc, ev, out, in_):
    if ev == "v":
        nc.vector.tensor_copy(out, in_)
    elif ev == "p":
        nc.gpsimd.tensor_copy(out, in_)
    else:
        nc.scalar.copy(out, in_)


def _attention_kernel(tc, out4, xTr, wqkvT, woutT, trimaskd, identd, vonesd):
    nc = tc.nc
    with ExitStack() as ctx:
        const = ctx.enter_context(tc.tile_pool(name="const", bufs=1))
        sbqk = ctx.enter_context(tc.tile_pool(name="sbqk", bufs=2))
        sbvv = ctx.enter_context(tc.tile_pool(name="sbvv", bufs=1))
        sbvt = ctx.enter_context(tc.tile_pool(name="sbvt", bufs=2))
        sbx = ctx.enter_context(tc.tile_pool(name="sbx", bufs=2))
        sbpt = ctx.enter_context(tc.tile_pool(name="sbpt", bufs=12))
        sba = ctx.enter_context(tc.tile_pool(name="sba", bufs=2))
        sbrc = ctx.enter_context(tc.tile_pool(name="sbrc", bufs=2))
        sbob = ctx.enter_context(tc.tile_pool(name="sbob", bufs=2))
        psS = ctx.enter_context(tc.tile_pool(name="psS", bufs=2, space="PSUM"))
        psAV = ctx.enter_context(tc.tile_pool(name="psAV", bufs=2, space="PSUM"))
        psOP = ctx.enter_context(tc.tile_pool(name="psOP", bufs=2, space="PSUM"))

        # ---- constants ----
        w_sb = const.tile([128, 8, 3 * DC], BF16, tag="wqkv")
        nc.sync.dma_start(out=w_sb, in_=wqkvT.rearrange("(dt p) f -> p dt f", p=128))
        wo_sb = const.tile([128, D], F32R, tag="wout")
        nc.sync.dma_start(out=wo_sb, in_=woutT)
        trimask = const.tile([128, 128], F32 if AV_DT == "r" else BF16,
                             tag="trimask")
        nc.sync.dma_start(out=trimask, in_=trimaskd)
        ident = const.tile([128, 128], F32R if AV_DT == "r" else BF16,
                           tag="ident")
        nc.sync.dma_start(out=ident, in_=identd)

        pools = (sbqk, sbvv, sbvt, sbx, sbpt, sba, sbrc, sbob, psS, psAV, psOP)

        # vv double-buffered manually (batch parity); ones cols written once
        # (memset on the idle Pool engine - keeps startup off the DMA path)
        # vv [128, kt, h, 128]: cols 0:64 V data (rewritten per batch), cols
        # 64:128 ones for the denominator row-sum trick (written once)
        vvs = []
        for pb in range(2):
            vv = sbvv.tile([128, TB, HPC, 128],
                           F32R if AV_DT == "r" else BF16, tag=f"vv{pb}")
            nc.sync.dma_start(out=vv[:, :, :, 64:128], in_=vonesd.rearrange(
                "p (t h c) -> p t h c", t=TB, h=HPC))
            vvs.append(vv)

        def body():
            _kernel_body(tc, out4, xTr, w_sb, wo_sb, trimask, ident, vvs, pools)

        nloop = int(os.environ.get("K_LOOP", "1"))
        if nloop > 1:
            with tc.For_i(0, nloop, 1):
                body()
        else:
            body()


def _kernel_body(tc, out4, xTr, w_sb, wo_sb, trimask, ident, vvs, pools):
    (sbqk, sbvv, sbvt, sbx, sbpt, sba, sbrc, sbob, psS, psAV, psOP) = pools
    nc = tc.nc

    # x chunk DMAs are issued one batch ahead so the SP queue isn't blocked
    # behind the previous batch's output DMAs when the next batch starts
    xts_tiles = {}

    def fetch_x(b):
        if b >= B:
            return
        for ci in range(NCH):
            tok0 = b * T + ci * CW
            xts = sbx.tile([128, 8, CW], BF16, tag="xts")
            nc.sync.dma_start(out=xts, in_=xTr[:, :, tok0:tok0 + CW])
            xts_tiles[(b, ci)] = xts

    fetch_x(0)
    for b in range(B):
        vv = vvs[b % 2]
        # ================= phase 1: QKV projection =================
        qT = sbqk.tile([128, T], F32R, tag="qT")
        kT = sbqk.tile([128, T], F32R, tag="kT")
        for ci in range(NCH):                    # 1024-token chunks
            xts = xts_tiles.pop((b, ci))
            for ft, dst in ((0, qT), (1, kT)):
                qkp = psS.tile([128, CW], F32, tag="s")
                for hf in range(2):              # matmul moving dim max 512
                    for dt in range(8):
                        nc.tensor.matmul(
                            qkp[:, hf * 512:(hf + 1) * 512],
                            w_sb[:, dt, ft * DC:(ft + 1) * DC],
                            xts[:, dt, hf * 512:(hf + 1) * 512],
                            start=(dt == 0), stop=(dt == 7),
                        )
                _copy(nc, QK_EV, dst[:, ci * CW:(ci + 1) * CW], qkp)
            # v in feature layout then transpose k-tiles of 128 toks
            vp = psS.tile([128, CW], F32, tag="s")
            for hf in range(2):
                for dt in range(8):
                    nc.tensor.matmul(
                        vp[:, hf * 512:(hf + 1) * 512],
                        w_sb[:, dt, 2 * DC:3 * DC],
                        xts[:, dt, hf * 512:(hf + 1) * 512],
                        start=(dt == 0), stop=(dt == 7),
                    )
            vdt = F32R if AV_DT == "r" else BF16
            vT = sbvt.tile([128, CW], vdt, tag="vT")
            _copy(nc, QK_EV, vT, vp)
            for k4 in range(2):
                trp = psOP.tile([128, 4, 128], vdt, tag="op")
                for ki in range(4):
                    kk = k4 * 4 + ki
                    nc.tensor.transpose(
                        trp[:, ki, :],
                        vT[:, kk * 128:(kk + 1) * 128],
                        ident,
                    )
                kt0 = ci * 8 + k4 * 4
                _copy(nc, V_EV,
                      vv[:, kt0:kt0 + 4, :, 0:64],
                      trp.rearrange("p a (h c) -> p a h c", h=HPC))

        fetch_x(b + 1)
        # ====== phase 2 + 3: attention per (chunk, head), fused outproj ====
        attnT = sba.tile([128, T], F32R, tag="attnT")
        for ci in range(NCH):
            nsub = ci * 8                        # fully-live k-tiles
            qs = ci * CW
            all_pts = []
            for h in range(HPC):
                hs = slice(h * 64, (h + 1) * 64)
                pts = []
                for kt in range(nsub):           # sub-diagonal k-tiles
                    sp = psS.tile([128, CW], F32, tag="s")
                    for hf in range(2):
                        nc.tensor.matmul(
                            sp[:, hf * 512:(hf + 1) * 512],
                            kT[hs, kt * 128:(kt + 1) * 128],
                            qT[hs, qs + hf * 512:qs + (hf + 1) * 512],
                            start=True, stop=True,
                        )
                    pt = sbpt.tile([128, CW],
                                   F32R if AV_DT == "r" else BF16, tag="p")
                    nc.scalar.activation(pt, sp, EXP, scale=SCALE)
                    pts.append((kt, pt, 0))
                for o in range(8):               # diagonal-band k-tiles
                    kt = nsub + o
                    off = 128 * o
                    sp = psS.tile([128, CW], F32, tag="s")
                    for c0 in range(off // 512 * 512, CW, 512):
                        o0 = max(off, c0)
                        nc.tensor.matmul(
                            sp[:, o0:c0 + 512],
                            kT[hs, kt * 128:(kt + 1) * 128],
                            qT[hs, qs + o0:qs + c0 + 512],
                            start=True, stop=True,
                        )
                    pt = sbpt.tile([128, CW],
                                   F32R if AV_DT == "r" else BF16, tag="p")
                    nc.scalar.activation(pt[:, off:CW], sp[:, off:CW],
                                         EXP, scale=SCALE)
                    eng = nc.gpsimd if MASK_EV == "p" else nc.vector
                    eng.tensor_tensor(
                        out=pt[:, off:off + 128], in0=pt[:, off:off + 128],
                        in1=trimask, op=mybir.AluOpType.mult)
                    pts.append((kt, pt, off))
                all_pts.append(pts)
            # AV in 512-halves so psav tiles stay one bank; norm right after
            # each half so the next head's AV can reuse the psAV ring
            for h in range(HPC):
                pts = all_pts[h]
                for half in range(2):
                    c0 = half * 512
                    contrib = [(kt, pt, max(off, c0)) for kt, pt, off in pts
                               if max(off, c0) < c0 + 512]
                    avp = psAV.tile([128, 512], F32, tag="av")
                    for i, (kt, pt, o0) in enumerate(contrib):
                        nc.tensor.matmul(
                            avp[:, o0 - c0:512],
                            vv[:, kt, h, :],
                            pt[:, o0:c0 + 512],
                            start=(i == 0), stop=(i == len(contrib) - 1),
                        )
                    rc = sbrc.tile([128, 512], F32, tag="rc")
                    nc.vector.reciprocal(rc[0:64, :], avp[64:128, :])
                    nc.vector.tensor_tensor(
                        out=attnT[h * 64:(h + 1) * 64, qs + c0:qs + c0 + 512],
                        in0=avp[0:64, :], in1=rc[0:64, :],
                        op=mybir.AluOpType.mult,
                    )
            # ---- phase 3: out projection for this chunk's two 512-blocks ---
            for half in range(2):
                qb = ci * 2 + half
                ob = sbob.tile([128, 4, D], BF16, tag="ob")
                for tl in range(4):
                    tt = qb * 4 + tl
                    for fc in range(2):
                        op_ = psOP.tile([128, 512], F32, tag="op")
                        nc.tensor.matmul(
                            op_,
                            attnT[:, tt * 128:(tt + 1) * 128],
                            wo_sb[:, fc * 512:(fc + 1) * 512],
                            start=True, stop=True,
                        )
                        ev = OB_EV if OB_EV != "b" else ("v" if fc == 0 else "s")
                        _copy(nc, ev, ob[:, tl, fc * 512:(fc + 1) * 512], op_)
                if OUTDMA_EV == "p":
                    nc.gpsimd.dma_start(out=out4[b * 4 + qb], in_=ob)
                else:
                    nc.sync.dma_start(out=out4[b * 4 + qb], in_=ob)


def build_module():
    nc = bacc.Bacc("TRN2", target_bir_lowering=False, debug=False,
                   num_devices=NCORES)
    xT = nc.declare_dram_parameter("xT", [D, TOK], BF16, isOutput=False)
    wqkvT = nc.declare_dram_parameter("wqkvT", [D, 3 * DC], BF16, isOutput=False)
    woutT = nc.declare_dram_parameter("woutT", [DC, D], F32R, isOutput=False)
    trimask = nc.declare_dram_parameter(
        "trimask", [128, 128], F32 if AV_DT == "r" else BF16, isOutput=False)
    ident = nc.declare_dram_parameter(
        "ident", [128, 128], F32R if AV_DT == "r" else BF16, isOutput=False)
    vones = nc.declare_dram_parameter(
        "vones", [128, TB * HPC * 64], F32R if AV_DT == "r" else BF16,
        isOutput=False)
    out = nc.declare_dram_parameter("out", [TOK, D], BF16, isOutput=True)
    with tile.TileContext(nc) as tc:
        _attention_kernel(
            tc,
            out[:].rearrange("(n tt p) d -> n p tt d", p=128, tt=4),
            xT[:].rearrange("(dt p) tok -> p dt tok", p=128),
            wqkvT[:], woutT[:], trimask[:], ident[:], vones[:],
        )
    nc.compile()
    return nc


def shard_inputs(x, w_qkv, w_out):
    """Returns per-core input maps (bf16 host-side prep)."""
    bf = ml_dtypes.bfloat16
    x_flat = np.asarray(x, np.float32).reshape(TOK, D)
    xT = np.ascontiguousarray(x_flat.T).astype(bf)       # [D, TOK]
    w_qkv = np.asarray(w_qkv, np.float32)
    w_out = np.asarray(w_out, np.float32)
    kp = np.arange(128)[:, None]
    qf = np.arange(128)[None, :]
    trimask = (kp <= qf).astype(
        np.float32 if AV_DT == "r" else bf)              # [128,128] lower-tri^T
    identm = np.eye(128, dtype=np.float32 if AV_DT == "r" else bf)
    vones = np.ones((128, TB * HPC * 64),
                    np.float32 if AV_DT == "r" else bf)
    in_maps = []
    for c in range(NCORES):
        r0 = c * DC
        wq = w_qkv[r0:r0 + DC]
        wk = w_qkv[D + r0:D + r0 + DC]
        wv = w_qkv[2 * D + r0:2 * D + r0 + DC]
        wqkvT = np.ascontiguousarray(
            np.concatenate([wq, wk, wv], axis=0).T).astype(bf)   # [D, 3*DC]
        woutT = np.ascontiguousarray(w_out[:, r0:r0 + DC].T)     # [DC, D] f32
        in_maps.append({"xT": xT, "wqkvT": wqkvT, "woutT": woutT,
                       "trimask": trimask, "ident": identm, "vones": vones})
    return in_maps


_NC_CACHE = None


def kernel(x, w_qkv, w_out):
    global _NC_CACHE, LAST_RESULTS
    if _NC_CACHE is None:
        _NC_CACHE = build_module()
    nc = _NC_CACHE
    in_maps = shard_inputs(x, w_qkv, w_out)
    os.environ["BASS_NEVER_TRACE"] = "1"
    res = run_bass_kernel_spmd(nc, in_maps, list(range(NCORES)), trace=False)
    LAST_RESULTS = res
    acc = np.zeros((TOK, D), dtype=np.float32)
    for r in res.results:
        acc += r["out"].astype(np.float32)
    return acc.reshape(B, T, D)
